# revision 1
# baseline (speedup 1.0000x reference)
"""Trainium2 Bass kernel for a GNN message-passing layer (GCL).

Reference computation:
    src = features[rows]; dst = features[cols]
    h = sigmoid(concat(src, dst) @ Wm1 + bm1)
    messages = softsign(h @ Wm2 + bm2)
    agg = segment_sum(messages, rows, N)
    g = sigmoid(concat(features, agg, time_embedding))
    g = sigmoid(g @ Wf1 + bf1)
    out = softsign(g @ Wf2 + bf2)

Restructure: concat(src, dst) @ Wm1 = A[rows] + B[cols] with A = X@Wm1[:F],
B = X@Wm1[F:] precomputed per node. All per-edge row movement is expressed
as one-hot expand matmuls on the tensor engine (256B-row DMA gathers are
descriptor-bound and infeasible at 650k rows):

  - edges are bucketed by (row-window w of 128 nodes, col-chunk c of CH
    nodes); one 128-edge tile per bucket, CH chosen so no bucket overflows;
  - S^T[m1, e] = A_w-expand + B_c-expand accumulated in PSUM via one-hot
    rhs matrices PR[n, e] and PC[p, e];
  - h^T = sigmoid(S^T + bm1)  (bias rides the partition dim for free);
  - msgs[e, m2] = (h^T-slice as lhsT) @ Wm2;  y = softsign(msgs);
  - aggT[m2, n] += (y as lhsT) @ P[e, n]  (the segment-sum).

Sharding: core k owns node range [k*1280, (k+1)*1280) (nodes padded to
10240) and all edges whose row lands there; each core computes the full B
table locally so no collectives are needed; outputs are concatenated on the
host. The per-core program is identical (SPMD); all per-core structure lives
in the input data (one-hot matrices, own-range feature slices).
"""

import numpy as np
import ml_dtypes

import concourse.bass as bass
import concourse.bacc as bacc
import concourse.mybir as mybir
import concourse.tile as tile
from concourse.bass_utils import run_bass_kernel_spmd
from concourse.masks import make_identity
from concourse.mybir import ActivationFunctionType as AF, AluOpType as ALU

BF16 = mybir.dt.bfloat16
F32 = mybir.dt.float32
NPBF16 = ml_dtypes.bfloat16

N = 10000
E = 640000
FD = 128
NCORES = 8
NPAD = 10240
NT = NPAD // 128         # 80 row windows total
NTC = NT // NCORES       # 10 windows per core
RANGE = NPAD // NCORES   # 1280 nodes per core
GROUP = 4                # tiles per elementwise batch (free dim 512)

# one-hot matrix dtype (BF16 safe; fp8 halves one-hot DMA traffic)
ONEHOT_DT = BF16
ONEHOT_NP = NPBF16

# softsign path: "act" (Abs+Reciprocal on ScalarE + mult on DVE), "dve"
# (copy+stt+add1+recip_approx+mult on DVE), or "split" (alternate groups).
SOFTSIGN_PATH = "act"


def _act_recip(nc, out_ap, in_ap, bias=1.0):
    """out = 1/(in + bias) on ScalarE. bass's activation() refuses
    func=Reciprocal wholesale; our denominator is >= 1 where the table is
    well behaved, so emit the instruction directly."""
    eng = nc.scalar
    ins = [
        eng.lower_ap(in_ap),
        mybir.ImmediateValue(dtype=F32, value=float(bias)),
        mybir.ImmediateValue(dtype=F32, value=1.0),
        mybir.ImmediateValue(dtype=F32, value=0.0),
    ]
    return eng.add_instruction(
        mybir.InstActivation(
            name=nc.get_next_instruction_name(),
            func=AF.Reciprocal,
            ins=ins,
            outs=[eng.lower_ap(out_ap)],
        )
    )


def _softsign_group(nc, pool, msgs_ap, y_ap, fd, act_path):
    """y = msgs / (1 + |msgs|); msgs in PSUM fp32, y -> SBUF bf16."""
    if act_path:
        a_ = pool.tile([128, fd], F32, tag="ss_a")
        r_ = pool.tile([128, fd], F32, tag="ss_r")
        nc.scalar.activation(a_[:], msgs_ap, AF.Abs)
        _act_recip(nc, r_[:], a_[:], bias=1.0)
        nc.vector.tensor_tensor(y_ap, msgs_ap, r_[:], ALU.mult)
    else:
        c_ = pool.tile([128, fd], F32, tag="ss_c")
        nc.vector.tensor_copy(c_[:], msgs_ap)
        a_ = pool.tile([128, fd], F32, tag="ss_a")
        nc.vector.scalar_tensor_tensor(a_[:], c_[:], -1.0, c_[:],
                                       ALU.mult, ALU.max)
        nc.vector.tensor_scalar_add(a_[:], a_[:], 1.0)
        r_ = pool.tile([128, fd], F32, tag="ss_r")
        nc.vector.reciprocal_approx_fast(r_[:], a_[:])
        nc.vector.tensor_tensor(y_ap, c_[:], r_[:], ALU.mult)


def build_program(CH: int, nonzero_bm2: bool, nonzero_bf2: bool) -> bass.Bass:
    """SPMD per-core program. CH = col-chunk node count (<=128)."""
    NCH = -(-NPAD // CH)                 # col chunks
    NCH_P = -(-NCH // GROUP) * GROUP     # padded per-window tile count
    NGW = NCH_P // GROUP                 # groups per window
    NGRP = NTC * NGW

    nc = bacc.Bacc("TRN2", debug=False, num_devices=NCORES)

    featn = nc.dram_tensor("featn", [NPAD, FD], BF16, kind="ExternalInput")
    ownfeat = nc.dram_tensor("ownfeat", [RANGE, FD], BF16, kind="ExternalInput")
    owntime = nc.dram_tensor("owntime", [RANGE, FD], BF16, kind="ExternalInput")
    p_oh = nc.dram_tensor("p_oh", [NGRP, 128, GROUP * 128], ONEHOT_DT,
                          kind="ExternalInput")
    pr_oh = nc.dram_tensor("pr_oh", [NGRP, 128, GROUP * 128], ONEHOT_DT,
                           kind="ExternalInput")
    pc_oh = nc.dram_tensor("pc_oh", [NGRP, 128, GROUP * 128], ONEHOT_DT,
                           kind="ExternalInput")
    wm1 = nc.dram_tensor("wm1", [128, 2 * FD], BF16, kind="ExternalInput")
    wm2 = nc.dram_tensor("wm2", [FD, FD], BF16, kind="ExternalInput")
    wf1 = nc.dram_tensor("wf1", [3 * FD, FD], BF16, kind="ExternalInput")
    wf2 = nc.dram_tensor("wf2", [FD, FD], BF16, kind="ExternalInput")
    bm1d = nc.dram_tensor("bm1", [FD], F32, kind="ExternalInput")
    bf1d = nc.dram_tensor("bf1", [FD], F32, kind="ExternalInput")
    if nonzero_bm2:
        bm2d = nc.dram_tensor("bm2", [FD], BF16, kind="ExternalInput")
    if nonzero_bf2:
        bf2d = nc.dram_tensor("bf2", [FD], BF16, kind="ExternalInput")
    outd = nc.dram_tensor("out", [RANGE, FD], F32, kind="ExternalOutput")

    # B for all nodes (+128 pad rows so chunk slices can overrun the end)
    B_dram = nc.dram_tensor("B_scratch", [NPAD + 128, FD], BF16)

    with tile.TileContext(nc) as tc:
        with (
            tc.tile_pool(name="const", bufs=1) as cst,
            tc.tile_pool(name="oh", bufs=3) as ohp,
            tc.tile_pool(name="hp", bufs=3) as hp,
            tc.tile_pool(name="grp", bufs=2) as grp,
            tc.tile_pool(name="abp", bufs=3) as abp,
            tc.tile_pool(name="ntp", bufs=2) as ntp,
            tc.tile_pool(name="ps_s", bufs=2, space="PSUM") as ps_s,
            tc.tile_pool(name="ps_m", bufs=2, space="PSUM") as ps_m,
            tc.tile_pool(name="ps_agg", bufs=2, space="PSUM") as ps_agg,
            tc.tile_pool(name="ps_misc", bufs=1, space="PSUM") as ps_misc,
        ):
            # ---- constants ----
            wm1_sb = cst.tile([128, 2 * FD], BF16)
            nc.sync.dma_start(out=wm1_sb[:], in_=wm1[:])
            wm2_sb = cst.tile([128, FD], BF16)
            nc.sync.dma_start(out=wm2_sb[:], in_=wm2[:])
            wf1_sb = cst.tile([128, 3 * FD], BF16)
            for c3 in range(3):
                nc.sync.dma_start(
                    out=wf1_sb[:, c3 * FD:(c3 + 1) * FD],
                    in_=wf1[c3 * FD:(c3 + 1) * FD, :],
                )
            wf2_sb = cst.tile([128, FD], BF16)
            nc.sync.dma_start(out=wf2_sb[:], in_=wf2[:])
            bm1_sb = cst.tile([128, 1], F32)
            nc.sync.dma_start(out=bm1_sb[:], in_=bm1d[:, None])
            bf1_sb = cst.tile([128, 1], F32)
            nc.sync.dma_start(out=bf1_sb[:], in_=bf1d[:, None])
            iden = cst.tile([128, 128], BF16)
            make_identity(nc, iden[:])
            if nonzero_bm2 or nonzero_bf2:
                ones_sb = cst.tile([1, 128], BF16)
                nc.gpsimd.memset(ones_sb[:], 1.0)
            if nonzero_bm2:
                bm2_sb = cst.tile([1, 128], BF16)
                nc.sync.dma_start(out=bm2_sb[:], in_=bm2d[None, :])
            if nonzero_bf2:
                bf2_sb = cst.tile([1, 128], BF16)
                nc.sync.dma_start(out=bf2_sb[:], in_=bf2d[None, :])

            # ---- phase AB: B (all nodes) -> DRAM ----
            for nt0 in range(NT):
                fn = abp.tile([128, 128], BF16, tag="fn")
                nc.sync.dma_start(
                    out=fn[:], in_=featn[nt0 * 128:(nt0 + 1) * 128, :]
                )
                ftp = ps_misc.tile([128, 128], BF16, tag="misc")
                nc.tensor.transpose(out=ftp[:], in_=fn[:], identity=iden[:])
                ft = abp.tile([128, 128], BF16, tag="ft")
                nc.vector.tensor_copy(ft[:], ftp[:])
                b_ps = ps_misc.tile([128, FD], F32, tag="misc")
                nc.tensor.matmul(b_ps[:], lhsT=ft[:], rhs=wm1_sb[:, FD:],
                                 start=True, stop=True)
                b_sb = abp.tile([128, FD], BF16, tag="absb")
                nc.scalar.copy(b_sb[:], b_ps[:])
                nc.sync.dma_start(
                    out=B_dram[nt0 * 128:(nt0 + 1) * 128, :], in_=b_sb[:]
                )

            # ---- own-range A (kept in SBUF) + gT1/gT3 ----
            A_sb = cst.tile([128, NTC * 128], BF16)   # [n, m1] per window
            gT1 = cst.tile([128, NTC * 128], BF16)
            gT3 = cst.tile([128, NTC * 128], BF16)
            for w in range(NTC):
                fn = abp.tile([128, 128], BF16, tag="fn")
                nc.sync.dma_start(
                    out=fn[:], in_=ownfeat[w * 128:(w + 1) * 128, :]
                )
                ftp = ps_misc.tile([128, 128], BF16, tag="misc")
                nc.tensor.transpose(out=ftp[:], in_=fn[:], identity=iden[:])
                ft = abp.tile([128, 128], BF16, tag="ft")
                nc.vector.tensor_copy(ft[:], ftp[:])
                nc.scalar.activation(gT1[:, w * 128:(w + 1) * 128], ftp[:],
                                     AF.Sigmoid)
                a_ps = ps_misc.tile([128, FD], F32, tag="misc")
                nc.tensor.matmul(a_ps[:], lhsT=ft[:], rhs=wm1_sb[:, :FD],
                                 start=True, stop=True)
                nc.scalar.copy(A_sb[:, w * 128:(w + 1) * 128], a_ps[:])

                tn = abp.tile([128, 128], BF16, tag="fn")
                nc.sync.dma_start(
                    out=tn[:], in_=owntime[w * 128:(w + 1) * 128, :]
                )
                ttp = ps_misc.tile([128, 128], BF16, tag="misc")
                nc.tensor.transpose(out=ttp[:], in_=tn[:], identity=iden[:])
                nc.scalar.activation(gT3[:, w * 128:(w + 1) * 128], ttp[:],
                                     AF.Sigmoid)

            # zero-fill B_dram's 128 pad rows (read by the last chunks'
            # lhsT slices; garbage NaNs would poison 0*NaN in the matmul)
            zb = abp.tile([128, 128], BF16, tag="fn")
            nc.gpsimd.memset(zb[:], 0)
            nc.sync.dma_start(out=B_dram[NPAD:NPAD + 128, :], in_=zb[:])

            # B table resident in SBUF: chunk c = B_dram[c*CH : c*CH+128]
            B_sb = cst.tile([128, NCH * 128], BF16)
            for c in range(NCH):
                nc.sync.dma_start(
                    out=B_sb[:, c * 128:(c + 1) * 128],
                    in_=B_dram[c * CH:c * CH + 128, :],
                )

            # ---- edge phase (w-major: one window's aggT at a time) ----
            for w in range(NTC):
                agg_ps = ps_agg.tile([128, 128], F32, tag="agg")
                for gw in range(NGW):
                    g = w * NGW + gw
                    s_ps = ps_s.tile([128, GROUP * 128], F32, tag="s")
                    # PR/PC one-hot loads for the group
                    pr_t = ohp.tile([128, GROUP * 128], ONEHOT_DT, tag="pr")
                    nc.sync.dma_start(out=pr_t[:], in_=pr_oh[g])
                    pc_t = ohp.tile([128, GROUP * 128], ONEHOT_DT, tag="pc")
                    nc.sync.dma_start(out=pc_t[:], in_=pc_oh[g])
                    for k in range(GROUP):
                        c = gw * GROUP + k
                        ks = slice(k * 128, (k + 1) * 128)
                        nc.tensor.matmul(
                            s_ps[:, ks],
                            lhsT=A_sb[:, w * 128:(w + 1) * 128],
                            rhs=pr_t[:, ks], start=True, stop=False,
                        )
                        cc = min(c, NCH - 1)
                        nc.tensor.matmul(
                            s_ps[:, ks],
                            lhsT=B_sb[:, cc * 128:(cc + 1) * 128],
                            rhs=pc_t[:, ks],
                            start=False, stop=True,
                        )
                    # h = sigmoid(S + bm1) for the whole group
                    h_t = hp.tile([128, GROUP * 128], BF16, tag="h")
                    nc.scalar.activation(h_t[:], s_ps[:], AF.Sigmoid,
                                         bias=bm1_sb[:])
                    # msgs = h @ Wm2 (+ bm2)
                    m_ps = ps_m.tile([128, GROUP * 128], F32, tag="m")
                    for k in range(GROUP):
                        ks = slice(k * 128, (k + 1) * 128)
                        if nonzero_bm2:
                            nc.tensor.matmul(
                                m_ps[:, ks], lhsT=ones_sb[:], rhs=bm2_sb[:],
                                start=True, stop=False)
                        nc.tensor.matmul(
                            m_ps[:, ks], lhsT=h_t[:, ks], rhs=wm2_sb[:],
                            start=not nonzero_bm2, stop=True,
                        )
                    # y = softsign(msgs)
                    y_t = hp.tile([128, GROUP * 128], BF16, tag="y")
                    if SOFTSIGN_PATH == "act":
                        act_path = True
                    elif SOFTSIGN_PATH == "dve":
                        act_path = False
                    else:
                        act_path = (g % 2 == 0)
                    _softsign_group(nc, grp, m_ps[:], y_t[:], GROUP * 128,
                                    act_path)
                    # scatter: aggT += y_k^T-expand
                    p_t = ohp.tile([128, GROUP * 128], ONEHOT_DT, tag="p")
                    nc.sync.dma_start(out=p_t[:], in_=p_oh[g])
                    for k in range(GROUP):
                        ks = slice(k * 128, (k + 1) * 128)
                        nc.tensor.matmul(
                            agg_ps[:], lhsT=y_t[:, ks], rhs=p_t[:, ks],
                            start=(gw == 0 and k == 0),
                            stop=(gw == NGW - 1 and k == GROUP - 1),
                        )

                # ---- node MLP for window w ----
                ws = slice(w * 128, (w + 1) * 128)
                gt2 = ntp.tile([128, 128], BF16, tag="gt2")
                nc.scalar.activation(gt2[:], agg_ps[:], AF.Sigmoid)
                g2_ps = ps_misc.tile([128, 128], F32, tag="misc")
                nc.tensor.matmul(g2_ps[:], lhsT=wf1_sb[:, :FD],
                                 rhs=gT1[:, ws], start=True, stop=False)
                nc.tensor.matmul(g2_ps[:], lhsT=wf1_sb[:, FD:2 * FD],
                                 rhs=gt2[:], start=False, stop=False)
                nc.tensor.matmul(g2_ps[:], lhsT=wf1_sb[:, 2 * FD:],
                                 rhs=gT3[:, ws], start=False, stop=True)
                g2_sb = ntp.tile([128, 128], BF16, tag="g2sb")
                nc.scalar.activation(g2_sb[:], g2_ps[:], AF.Sigmoid,
                                     bias=bf1_sb[:])
                o_ps = ps_misc.tile([128, 128], F32, tag="misc")
                if nonzero_bf2:
                    nc.tensor.matmul(o_ps[:], lhsT=ones_sb[:], rhs=bf2_sb[:],
                                     start=True, stop=False)
                nc.tensor.matmul(o_ps[:], lhsT=g2_sb[:], rhs=wf2_sb[:],
                                 start=not nonzero_bf2, stop=True)
                # softsign in fp32 on DVE (output precision matters here)
                oc = ntp.tile([128, 128], F32, tag="oc")
                nc.vector.tensor_copy(oc[:], o_ps[:])
                oa = ntp.tile([128, 128], F32, tag="oa")
                nc.vector.scalar_tensor_tensor(oa[:], oc[:], -1.0, oc[:],
                                               ALU.mult, ALU.max)
                nc.vector.tensor_scalar_add(oa[:], oa[:], 1.0)
                orr = ntp.tile([128, 128], F32, tag="orr")
                nc.vector.reciprocal_approx_fast(orr[:], oa[:])
                oy = ntp.tile([128, 128], F32, tag="oy")
                nc.vector.tensor_tensor(oy[:], oc[:], orr[:], ALU.mult)
                nc.sync.dma_start(out=outd[ws, :], in_=oy[:])

    nc.compile()
    return nc


def choose_chunk(rs, cs):
    """Largest CH in {128,120,...,64} with every (window, chunk) bucket
    <= 128 edges, checked over the actual data (global windows cover all
    cores at once)."""
    w_glob = rs // 128
    for CH in (128, 120, 112, 104, 96, 88, 80, 72, 64):
        nch = -(-NPAD // CH)
        bid = w_glob * nch + cs // CH
        if np.bincount(bid).max() <= 128:
            return CH
    raise ValueError("no feasible col-chunk size; graph too skewed")


def prepare_inputs(features, rows, cols, time_embedding,
                   Wm1, bm1, Wm2, bm2, Wf1, bf1, Wf2, bf2):
    features = np.asarray(features, np.float32)
    time_embedding = np.asarray(time_embedding, np.float32)
    rows = np.asarray(rows).astype(np.int64)
    cols = np.asarray(cols).astype(np.int64)
    Wm1 = np.asarray(Wm1, np.float32)
    Wm2 = np.asarray(Wm2, np.float32)
    Wf1 = np.asarray(Wf1, np.float32)
    Wf2 = np.asarray(Wf2, np.float32)
    bm1 = np.asarray(bm1, np.float32).reshape(FD)
    bm2 = np.asarray(bm2, np.float32).reshape(FD)
    bf1 = np.asarray(bf1, np.float32).reshape(FD)
    bf2 = np.asarray(bf2, np.float32).reshape(FD)

    CH = choose_chunk(rows, cols)
    NCH = -(-NPAD // CH)
    NCH_P = -(-NCH // GROUP) * GROUP
    NGW = NCH_P // GROUP
    TT_P = NTC * NCH_P
    NGRP = NTC * NGW

    feat_pad = np.zeros((NPAD, FD), np.float32)
    feat_pad[:N] = features
    time_pad = np.zeros((NPAD, FD), np.float32)
    time_pad[:N] = time_embedding
    featbf = feat_pad.astype(NPBF16)
    timebf = time_pad.astype(NPBF16)
    wm1cat = np.concatenate([Wm1[:FD], Wm1[FD:]], axis=1).astype(NPBF16)

    nonzero_bm2 = bool(np.any(bm2))
    nonzero_bf2 = bool(np.any(bf2))
    common = {
        "featn": featbf,
        "wm1": wm1cat, "wm2": Wm2.astype(NPBF16),
        "wf1": Wf1.astype(NPBF16), "wf2": Wf2.astype(NPBF16),
        "bm1": bm1, "bf1": bf1,
    }
    if nonzero_bm2:
        common["bm2"] = bm2.astype(NPBF16)
    if nonzero_bf2:
        common["bf2"] = bf2.astype(NPBF16)

    in_maps = []
    for core in range(NCORES):
        base = core * RANGE
        sel = (rows >= base) & (rows < base + RANGE)
        r_c = rows[sel]
        c_c = cols[sel]
        w_loc = (r_c - base) // 128
        cch = c_c // CH
        tid = w_loc * NCH_P + cch        # window-major tile order
        order = np.argsort(tid, kind="stable")
        r_s, c_s, t_s = r_c[order], c_c[order], tid[order]
        # slot within tile
        slot = np.zeros(len(t_s), np.int64)
        if len(t_s):
            newt = np.r_[True, t_s[1:] != t_s[:-1]]
            starts = np.nonzero(newt)[0]
            slot = np.arange(len(t_s)) - np.repeat(starts, np.diff(
                np.r_[starts, len(t_s)]))
        assert slot.max(initial=0) < 128, "bucket overflow"
        epos = t_s * 128 + slot          # edge position in tile grid
        rrel = (r_c[order] - base) % 128
        crel = c_s - (t_s % NCH_P) * CH

        P = np.zeros((TT_P * 128, 128), np.float32)
        P[epos, rrel] = 1.0
        PC = np.zeros((TT_P * 128, 128), np.float32)
        PC[epos, crel] = 1.0

        def pack(M, transpose):
            M4 = M.reshape(NGRP, GROUP, 128, 128)
            if transpose:
                M4 = M4.transpose(0, 3, 1, 2)     # [g, n, k, e]
            else:
                M4 = M4.transpose(0, 2, 1, 3)     # [g, e, k, n]
            return np.ascontiguousarray(
                M4.reshape(NGRP, 128, GROUP * 128).astype(ONEHOT_NP)
            )

        m = dict(common)
        m["p_oh"] = pack(P, False)     # [e, n] per tile
        m["pr_oh"] = pack(P, True)     # [n, e] per tile
        m["pc_oh"] = pack(PC, True)    # [p, e] per tile
        m["ownfeat"] = featbf[base:base + RANGE]
        m["owntime"] = timebf[base:base + RANGE]
        in_maps.append(m)

    return CH, nonzero_bm2, nonzero_bf2, in_maps


def kernel(features, rows, cols, time_embedding,
           Wm1, bm1, Wm2, bm2, Wf1, bf1, Wf2, bf2) -> np.ndarray:
    CH, nz_bm2, nz_bf2, in_maps = prepare_inputs(
        features, rows, cols, time_embedding,
        Wm1, bm1, Wm2, bm2, Wf1, bf1, Wf2, bf2,
    )
    nc = build_program(CH, nz_bm2, nz_bf2)
    res = run_bass_kernel_spmd(nc, in_maps, list(range(NCORES)))
    out = np.concatenate(
        [res.results[c]["out"] for c in range(NCORES)], axis=0
    )[:N]
    return np.ascontiguousarray(out.astype(np.float32))



# revision 5
# speedup vs baseline: 1.5202x; 1.5202x over previous
"""Trainium2 Bass kernel for a GNN message-passing layer (GCL).

Reference computation:
    src = features[rows]; dst = features[cols]
    h = sigmoid(concat(src, dst) @ Wm1 + bm1)
    messages = softsign(h @ Wm2 + bm2)
    agg = segment_sum(messages, rows, N)
    g = sigmoid(concat(features, agg, time_embedding))
    g = sigmoid(g @ Wf1 + bf1)
    out = softsign(g @ Wf2 + bf2)

Restructure: concat(src, dst) @ Wm1 = A[rows] + B[cols] with A = X@Wm1[:F],
B = X@Wm1[F:] precomputed per node. All per-edge row movement is expressed
as one-hot expand matmuls on the tensor engine (256B-row DMA gathers are
descriptor-bound and infeasible at 650k rows):

  - edges are bucketed by (row-window w of 128 nodes, col-chunk c of CH
    nodes); one 128-edge tile per bucket, CH chosen so no bucket overflows;
  - S^T[m1, e] = A_w-expand + B_c-expand accumulated in PSUM via one-hot
    rhs matrices PR[n, e] and PC[p, e] (fp8, rhs-only operands);
  - h^T = sigmoid(S^T + bm1)  (bias rides the partition dim for free);
  - msgs[e, m2] = (h^T-slice as lhsT) @ Wm2;  y = softsign(msgs) via two
    custom DVE uops (seed + NR1, then NR2 * m) straight from PSUM fp32;
  - aggT[m2, n] += (y as lhsT) @ P[e, n]  (the segment-sum).

Perf notes vs the naive layout (all verified against the TimelineSim cost
model and HW):
  - softsign on the scalar engine thrashes the activation-function table
    (sigmoid and reciprocal live in different sets; 2 reloads x 1.28us per
    group) -- the custom DVE ops keep ScalarE on the sigmoid set only;
  - HWDGE descriptor generation costs a fixed ~625ns per DMA instruction,
    so one-hot loads are super-batched SUPER groups per dma_start;
  - features/time arrive host-transposed ([f, node]) so the A/B tables are
    plain matmuls (no on-chip transposes); B's overlapping 128-row chunk
    views are assembled on-chip with shift-matmuls against a sliced
    identity (no DRAM round-trip).

Sharding: core k owns node range [k*1280, (k+1)*1280) (nodes padded to
10240) and all edges whose row lands there; each core computes the full B
table locally so no collectives are needed; outputs are concatenated on the
host. The per-core program is identical (SPMD); all per-core structure lives
in the input data (one-hot matrices, own-range feature slices).
"""

import re

import numpy as np
import ml_dtypes

import concourse.bass as bass
import concourse.bacc as bacc
import concourse.mybir as mybir
import concourse.tile as tile
import concourse.dve_ops as dvo
from concourse.bass_utils import run_bass_kernel_spmd
from concourse.dve_ops import DveOp, RECIP_APPROX_FAST_CONSTS as _RC
from concourse.dve_spec import Spec, Src0, Src1, C0, C1, Zero, One, Bin, maxx
from concourse.dve_uop import AluOp as _UopAlu
from concourse.masks import make_identity
from concourse.mybir import ActivationFunctionType as AF

BF16 = mybir.dt.bfloat16
F32 = mybir.dt.float32
FP8 = mybir.dt.float8e4
NPBF16 = ml_dtypes.bfloat16
NPFP8 = ml_dtypes.float8_e4m3

N = 10000
E = 640000
FD = 128
NCORES = 8
NPAD = 10240
NT = NPAD // 128         # 80 row windows total
NTC = NT // NCORES       # 10 windows per core
RANGE = NPAD // NCORES   # 1280 nodes per core
GROUP = 4                # tiles per elementwise batch (free dim 512)
SUPER = 5                # groups per one-hot dma_start

ONEHOT_DT = FP8
ONEHOT_NP = NPFP8


# ---- custom DVE softsign: y = m / (1 + |m|) in two uops ----------------
# op1: t = 1+|m|; seed = ~t * c0 (exponent-flip); one Newton pass.
# op2: second Newton pass fused with the final multiply by m.
def _np_ss_seed(m, c0, c1):
    t = 1.0 + np.abs(m.astype(np.float32))
    nt = (~t.view(np.int32)).view(np.float32)
    y0 = nt * c0
    return y0 * (c1 - t * y0)


def _ref_ss_seed(in0, in1, s0, s1, imm2):
    return _np_ss_seed(in0, s0, s1)


def _ref_ss_fin(in0, in1, s0, s1, imm2):
    t = 1.0 + np.abs(in0.astype(np.float32))
    return in0 * (in1 * (s0 - t * in1))


_t1 = maxx(Src0, Zero - Src0) + One
_nt1 = Bin(_UopAlu.BITWISE_NOT, _t1, _t1)
_sy0 = _nt1 * C0
SS_SEED = DveOp(
    "SOFTSIGN_RECIP_SEED",
    Spec(body=_sy0 * (C1 - _t1 * _sy0), reference=_ref_ss_seed),
    subdim=False, uops_sha={},
)
_t2 = maxx(Src0, Zero - Src0) + One
SS_FIN = DveOp(
    "SOFTSIGN_FINISH",
    Spec(body=Src0 * (Src1 * (C0 - _t2 * Src1)), reference=_ref_ss_fin),
    subdim=False, uops_sha={},
)


def _register_dve_op(op):
    if op.name not in dvo._SUB_OPCODE_FOR_NAME:
        dvo.OPS.append(op)
        dvo._SUB_OPCODE_FOR_NAME[op.name] = (
            dvo._CUSTOM_DVE_ROW_BASE + len(dvo.OPS) - 1
        )
        dvo.CUSTOM_DVE_SPECS[op.name] = op.spec
    try:
        op.compile("v3")
    except ValueError as e:
        m = re.search(r"v3: ([0-9a-f]+)", str(e))
        if not m:
            raise
        op.uops_sha["v3"] = m.group(1)
    op.compile("v3")


_register_dve_op(SS_SEED)
_register_dve_op(SS_FIN)


def _softsign(nc, pool, m_ap, y_ap, fd, tag):
    """y = m / (1 + |m|); m in PSUM fp32, y -> SBUF (any dtype)."""
    y1 = pool.tile([128, fd], F32, tag=f"ss1_{tag}")
    nc.vector._custom_dve(SS_SEED, out=y1[:], in0=m_ap,
                          s0=_RC["s0"], s1=_RC["s1"])
    nc.vector._custom_dve(SS_FIN, out=y_ap, in0=m_ap, in1=y1[:], s0=2.0)


def build_program(CH: int, nonzero_bm2: bool, nonzero_bf2: bool) -> bass.Bass:
    """SPMD per-core program. CH = col-chunk node count (<=128)."""
    NCH = -(-NPAD // CH)                 # col chunks
    NCH_P = -(-NCH // GROUP) * GROUP     # padded per-window tile count
    NGW = NCH_P // GROUP                 # groups per window
    NGRP = NTC * NGW
    NSG = -(-NGRP // SUPER)              # super groups (one-hot dma batches)

    nc = bacc.Bacc("TRN2", debug=False, num_devices=NCORES)

    featT = nc.dram_tensor("featT", [128, NPAD], BF16, kind="ExternalInput")
    ownfeatT = nc.dram_tensor("ownfeatT", [128, RANGE], BF16,
                              kind="ExternalInput")
    owntimeT = nc.dram_tensor("owntimeT", [128, RANGE], BF16,
                              kind="ExternalInput")
    p_oh = nc.dram_tensor("p_oh", [NSG, 128, SUPER * GROUP * 128], ONEHOT_DT,
                          kind="ExternalInput")
    pr_oh = nc.dram_tensor("pr_oh", [NSG, 128, SUPER * GROUP * 128],
                           ONEHOT_DT, kind="ExternalInput")
    pc_oh = nc.dram_tensor("pc_oh", [NSG, 128, SUPER * GROUP * 128],
                           ONEHOT_DT, kind="ExternalInput")
    wm1 = nc.dram_tensor("wm1", [128, 2 * FD], BF16, kind="ExternalInput")
    wm2 = nc.dram_tensor("wm2", [FD, FD], BF16, kind="ExternalInput")
    wf1 = nc.dram_tensor("wf1", [3 * FD, FD], BF16, kind="ExternalInput")
    wf2 = nc.dram_tensor("wf2", [FD, FD], BF16, kind="ExternalInput")
    bm1d = nc.dram_tensor("bm1", [FD], F32, kind="ExternalInput")
    bf1d = nc.dram_tensor("bf1", [FD], F32, kind="ExternalInput")
    if nonzero_bm2:
        bm2d = nc.dram_tensor("bm2", [FD], BF16, kind="ExternalInput")
    if nonzero_bf2:
        bf2d = nc.dram_tensor("bf2", [FD], BF16, kind="ExternalInput")
    outd = nc.dram_tensor("out", [NTC, 128, FD], F32, kind="ExternalOutput")

    with tile.TileContext(nc) as tc:
        with (
            tc.tile_pool(name="const", bufs=1) as cst,
            tc.tile_pool(name="oh", bufs=2) as ohp,
            tc.tile_pool(name="hp", bufs=3) as hp,
            tc.tile_pool(name="grp", bufs=2) as grp,
            tc.tile_pool(name="abp", bufs=2) as abp,
            tc.tile_pool(name="ntp", bufs=2) as ntp,
            tc.tile_pool(name="ps_s", bufs=2, space="PSUM") as ps_s,
            tc.tile_pool(name="ps_m", bufs=2, space="PSUM") as ps_m,
            tc.tile_pool(name="ps_agg", bufs=2, space="PSUM") as ps_agg,
            tc.tile_pool(name="ps_misc", bufs=1, space="PSUM") as ps_misc,
        ):
            # ---- constants ----
            featT_sb = cst.tile([128, NPAD], BF16)
            nc.sync.dma_start(out=featT_sb[:], in_=featT[:])
            owntimeT_sb = cst.tile([128, RANGE], BF16)
            nc.sync.dma_start(out=owntimeT_sb[:], in_=owntimeT[:])
            wm1_sb = cst.tile([128, 2 * FD], BF16)
            nc.sync.dma_start(out=wm1_sb[:], in_=wm1[:])
            wm2_sb = cst.tile([128, FD], BF16)
            nc.sync.dma_start(out=wm2_sb[:], in_=wm2[:])
            wf1_sb = cst.tile([128, 3 * FD], BF16)
            for c3 in range(3):
                nc.sync.dma_start(
                    out=wf1_sb[:, c3 * FD:(c3 + 1) * FD],
                    in_=wf1[c3 * FD:(c3 + 1) * FD, :],
                )
            wf2_sb = cst.tile([128, FD], BF16)
            nc.sync.dma_start(out=wf2_sb[:], in_=wf2[:])
            bm1_sb = cst.tile([128, 1], F32)
            nc.sync.dma_start(out=bm1_sb[:], in_=bm1d[:, None])
            bf1_sb = cst.tile([128, 1], F32)
            nc.sync.dma_start(out=bf1_sb[:], in_=bf1d[:, None])
            iden = cst.tile([128, 128], BF16)
            make_identity(nc, iden[:])
            if nonzero_bm2 or nonzero_bf2:
                ones_sb = cst.tile([1, 128], BF16)
                nc.gpsimd.memset(ones_sb[:], 1.0)
            if nonzero_bm2:
                bm2_sb = cst.tile([1, 128], BF16)
                nc.sync.dma_start(out=bm2_sb[:], in_=bm2d[None, :])
            if nonzero_bf2:
                bf2_sb = cst.tile([1, 128], BF16)
                nc.sync.dma_start(out=bf2_sb[:], in_=bf2d[None, :])

            # ---- B table for all nodes, aligned 128-node tiles ----
            # B_full[:, t*128+p] = B[t*128+p, m1-col p'... [node-in-tile, m1]
            B_full = cst.tile([128, NT * 128], BF16)
            for t in range(NT):
                b_ps = ps_misc.tile([128, FD], F32, tag="misc")
                nc.tensor.matmul(
                    b_ps[:], lhsT=featT_sb[:, t * 128:(t + 1) * 128],
                    rhs=wm1_sb[:, FD:], start=True, stop=True,
                )
                nc.vector.tensor_copy(B_full[:, t * 128:(t + 1) * 128],
                                      b_ps[:])

            # chunk views B_sb[:, c] = B[c*CH : c*CH+128] via shift-matmuls:
            # lhsT = D1[:, o:o+128] maps tile-t0 row o+i -> out row i; D2
            # slice brings tile t0+1 rows into the tail. Column slices only
            # (partition offsets are restricted to 0/32/64 on the PE).
            D1 = cst.tile([128, 256], BF16)
            nc.gpsimd.memset(D1[:], 0)
            make_identity(nc, D1[:, 0:128])
            D2 = cst.tile([128, 256], BF16)
            nc.gpsimd.memset(D2[:], 0)
            make_identity(nc, D2[:, 128:256])
            B_sb = cst.tile([128, NCH * 128], BF16)
            for c in range(NCH):
                base = c * CH
                t0 = base // 128
                o = base - t0 * 128
                cs = slice(c * 128, (c + 1) * 128)
                ch_ps = ps_misc.tile([128, 128], F32, tag="misc")
                two = o != 0 and t0 + 1 < NT
                nc.tensor.matmul(
                    ch_ps[:], lhsT=D1[:, o:o + 128],
                    rhs=B_full[:, t0 * 128:(t0 + 1) * 128],
                    start=True, stop=not two,
                )
                if two:
                    nc.tensor.matmul(
                        ch_ps[:], lhsT=D2[:, o:o + 128],
                        rhs=B_full[:, (t0 + 1) * 128:(t0 + 2) * 128],
                        start=False, stop=True,
                    )
                nc.vector.tensor_copy(B_sb[:, cs], ch_ps[:])

            # ---- own-range A (kept in SBUF) + gT1/gT3 ----
            A_sb = cst.tile([128, NTC * 128], BF16)   # [n, m1] per window
            gT1 = cst.tile([128, NTC * 128], BF16)
            gT3 = cst.tile([128, NTC * 128], BF16)
            ownfeatT_sb = cst.tile([128, RANGE], BF16)
            nc.sync.dma_start(out=ownfeatT_sb[:], in_=ownfeatT[:])
            for w in range(NTC):
                ws = slice(w * 128, (w + 1) * 128)
                a_ps = ps_misc.tile([128, FD], F32, tag="misc")
                nc.tensor.matmul(a_ps[:], lhsT=ownfeatT_sb[:, ws],
                                 rhs=wm1_sb[:, :FD], start=True, stop=True)
                nc.scalar.copy(A_sb[:, ws], a_ps[:])
            nc.scalar.activation(gT1[:], ownfeatT_sb[:], AF.Sigmoid)
            nc.scalar.activation(gT3[:], owntimeT_sb[:], AF.Sigmoid)

            # ---- edge phase (w-major: one window's aggT at a time) ----
            sg_tiles = {}

            def oh_tile(which, dram, g):
                """SBUF slice for group g of one-hot stream `which`."""
                sg, off = divmod(g, SUPER)
                key = (which, sg)
                if key not in sg_tiles:
                    t_ = ohp.tile([128, SUPER * GROUP * 128], ONEHOT_DT,
                                  tag=f"oh_{which}")
                    nc.sync.dma_start(out=t_[:], in_=dram[sg])
                    sg_tiles[key] = t_
                t_ = sg_tiles[key]
                return t_[:, off * GROUP * 128:(off + 1) * GROUP * 128]

            for w in range(NTC):
                agg_ps = ps_agg.tile([128, 128], F32, tag="agg")
                for gw in range(NGW):
                    g = w * NGW + gw
                    s_ps = ps_s.tile([128, GROUP * 128], F32, tag="s")
                    pr_t = oh_tile("pr", pr_oh, g)
                    pc_t = oh_tile("pc", pc_oh, g)
                    for k in range(GROUP):
                        c = gw * GROUP + k
                        ks = slice(k * 128, (k + 1) * 128)
                        nc.tensor.matmul(
                            s_ps[:, ks],
                            lhsT=A_sb[:, w * 128:(w + 1) * 128],
                            rhs=pr_t[:, ks], start=True, stop=False,
                        )
                        cc = min(c, NCH - 1)
                        nc.tensor.matmul(
                            s_ps[:, ks],
                            lhsT=B_sb[:, cc * 128:(cc + 1) * 128],
                            rhs=pc_t[:, ks],
                            start=False, stop=True,
                        )
                    # h = sigmoid(S + bm1) for the whole group
                    h_t = hp.tile([128, GROUP * 128], BF16, tag="h")
                    nc.scalar.activation(h_t[:], s_ps[:], AF.Sigmoid,
                                         bias=bm1_sb[:])
                    # msgs = h @ Wm2 (+ bm2)
                    m_ps = ps_m.tile([128, GROUP * 128], F32, tag="m")
                    for k in range(GROUP):
                        ks = slice(k * 128, (k + 1) * 128)
                        if nonzero_bm2:
                            nc.tensor.matmul(
                                m_ps[:, ks], lhsT=ones_sb[:], rhs=bm2_sb[:],
                                start=True, stop=False)
                        nc.tensor.matmul(
                            m_ps[:, ks], lhsT=h_t[:, ks], rhs=wm2_sb[:],
                            start=not nonzero_bm2, stop=True,
                        )
                    # y = softsign(msgs)
                    y_t = hp.tile([128, GROUP * 128], BF16, tag="y")
                    _softsign(nc, grp, m_ps[:], y_t[:], GROUP * 128, "e")
                    # scatter: aggT += y_k^T-expand
                    p_t = oh_tile("p", p_oh, g)
                    for k in range(GROUP):
                        ks = slice(k * 128, (k + 1) * 128)
                        nc.tensor.matmul(
                            agg_ps[:], lhsT=y_t[:, ks], rhs=p_t[:, ks],
                            start=(gw == 0 and k == 0),
                            stop=(gw == NGW - 1 and k == GROUP - 1),
                        )

                # ---- node MLP for window w ----
                ws = slice(w * 128, (w + 1) * 128)
                gt2 = ntp.tile([128, 128], BF16, tag="gt2")
                nc.scalar.activation(gt2[:], agg_ps[:], AF.Sigmoid)
                g2_ps = ps_misc.tile([128, 128], F32, tag="misc")
                nc.tensor.matmul(g2_ps[:], lhsT=wf1_sb[:, :FD],
                                 rhs=gT1[:, ws], start=True, stop=False)
                nc.tensor.matmul(g2_ps[:], lhsT=wf1_sb[:, FD:2 * FD],
                                 rhs=gt2[:], start=False, stop=False)
                nc.tensor.matmul(g2_ps[:], lhsT=wf1_sb[:, 2 * FD:],
                                 rhs=gT3[:, ws], start=False, stop=True)
                g2_sb = ntp.tile([128, 128], BF16, tag="g2sb")
                nc.scalar.activation(g2_sb[:], g2_ps[:], AF.Sigmoid,
                                     bias=bf1_sb[:])
                o_ps = ps_misc.tile([128, 128], F32, tag="misc")
                if nonzero_bf2:
                    nc.tensor.matmul(o_ps[:], lhsT=ones_sb[:], rhs=bf2_sb[:],
                                     start=True, stop=False)
                nc.tensor.matmul(o_ps[:], lhsT=g2_sb[:], rhs=wf2_sb[:],
                                 start=not nonzero_bf2, stop=True)
                # softsign in fp32 (output precision matters here)
                oy = ntp.tile([128, 128], F32, tag="oy")
                _softsign(nc, ntp, o_ps[:], oy[:], 128, "n")
                nc.sync.dma_start(out=outd[w], in_=oy[:])

    nc.compile()
    return nc


def choose_chunk(rs, cs):
    """Largest CH in {128,120,...,64} with every (window, chunk) bucket
    <= 128 edges, checked over the actual data (global windows cover all
    cores at once)."""
    w_glob = rs // 128
    for CH in (128, 120, 112, 104, 96, 88, 80, 72, 64):
        nch = -(-NPAD // CH)
        bid = w_glob * nch + cs // CH
        if np.bincount(bid).max() <= 128:
            return CH
    raise ValueError("no feasible col-chunk size; graph too skewed")


def prepare_inputs(features, rows, cols, time_embedding,
                   Wm1, bm1, Wm2, bm2, Wf1, bf1, Wf2, bf2):
    features = np.asarray(features, np.float32)
    time_embedding = np.asarray(time_embedding, np.float32)
    rows = np.asarray(rows).astype(np.int64)
    cols = np.asarray(cols).astype(np.int64)
    Wm1 = np.asarray(Wm1, np.float32)
    Wm2 = np.asarray(Wm2, np.float32)
    Wf1 = np.asarray(Wf1, np.float32)
    Wf2 = np.asarray(Wf2, np.float32)
    bm1 = np.asarray(bm1, np.float32).reshape(FD)
    bm2 = np.asarray(bm2, np.float32).reshape(FD)
    bf1 = np.asarray(bf1, np.float32).reshape(FD)
    bf2 = np.asarray(bf2, np.float32).reshape(FD)

    CH = choose_chunk(rows, cols)
    NCH = -(-NPAD // CH)
    NCH_P = -(-NCH // GROUP) * GROUP
    NGW = NCH_P // GROUP
    TT_P = NTC * NCH_P
    NGRP = NTC * NGW
    NSG = -(-NGRP // SUPER)

    feat_pad = np.zeros((NPAD, FD), np.float32)
    feat_pad[:N] = features
    time_pad = np.zeros((NPAD, FD), np.float32)
    time_pad[:N] = time_embedding
    featTbf = np.ascontiguousarray(feat_pad.T.astype(NPBF16))
    timeTbf = np.ascontiguousarray(time_pad.T.astype(NPBF16))
    wm1cat = np.concatenate([Wm1[:FD], Wm1[FD:]], axis=1).astype(NPBF16)

    nonzero_bm2 = bool(np.any(bm2))
    nonzero_bf2 = bool(np.any(bf2))
    common = {
        "featT": featTbf,
        "wm1": wm1cat, "wm2": Wm2.astype(NPBF16),
        "wf1": Wf1.astype(NPBF16), "wf2": Wf2.astype(NPBF16),
        "bm1": bm1, "bf1": bf1,
    }
    if nonzero_bm2:
        common["bm2"] = bm2.astype(NPBF16)
    if nonzero_bf2:
        common["bf2"] = bf2.astype(NPBF16)

    in_maps = []
    for core in range(NCORES):
        base = core * RANGE
        sel = (rows >= base) & (rows < base + RANGE)
        r_c = rows[sel]
        c_c = cols[sel]
        w_loc = (r_c - base) // 128
        cch = c_c // CH
        tid = w_loc * NCH_P + cch        # window-major tile order
        order = np.argsort(tid, kind="stable")
        r_s, c_s, t_s = r_c[order], c_c[order], tid[order]
        # slot within tile
        slot = np.zeros(len(t_s), np.int64)
        if len(t_s):
            newt = np.r_[True, t_s[1:] != t_s[:-1]]
            starts = np.nonzero(newt)[0]
            slot = np.arange(len(t_s)) - np.repeat(starts, np.diff(
                np.r_[starts, len(t_s)]))
        assert slot.max(initial=0) < 128, "bucket overflow"
        epos = t_s * 128 + slot          # edge position in tile grid
        rrel = (r_c[order] - base) % 128
        crel = c_s - (t_s % NCH_P) * CH

        P = np.zeros((TT_P * 128, 128), np.float32)
        P[epos, rrel] = 1.0
        PC = np.zeros((TT_P * 128, 128), np.float32)
        PC[epos, crel] = 1.0

        def pack(M, transpose):
            M4 = M.reshape(NGRP, GROUP, 128, 128)
            if transpose:
                M4 = M4.transpose(0, 3, 1, 2)     # [g, n, k, e]
            else:
                M4 = M4.transpose(0, 2, 1, 3)     # [g, e, k, n]
            M4 = M4.reshape(NGRP, 128, GROUP * 128)
            # pad groups up to SUPER multiple, then super-batch
            pad = NSG * SUPER - NGRP
            if pad:
                M4 = np.concatenate(
                    [M4, np.zeros((pad, 128, GROUP * 128), M4.dtype)], axis=0)
            M5 = M4.reshape(NSG, SUPER, 128, GROUP * 128)
            M5 = M5.transpose(0, 2, 1, 3).reshape(
                NSG, 128, SUPER * GROUP * 128)
            return np.ascontiguousarray(M5.astype(ONEHOT_NP))

        m = dict(common)
        m["p_oh"] = pack(P, False)     # [e, n] per tile
        m["pr_oh"] = pack(P, True)     # [n, e] per tile
        m["pc_oh"] = pack(PC, True)    # [p, e] per tile
        m["ownfeatT"] = np.ascontiguousarray(featTbf[:, base:base + RANGE])
        m["owntimeT"] = np.ascontiguousarray(timeTbf[:, base:base + RANGE])
        in_maps.append(m)

    return CH, nonzero_bm2, nonzero_bf2, in_maps


def kernel(features, rows, cols, time_embedding,
           Wm1, bm1, Wm2, bm2, Wf1, bf1, Wf2, bf2) -> np.ndarray:
    CH, nz_bm2, nz_bf2, in_maps = prepare_inputs(
        features, rows, cols, time_embedding,
        Wm1, bm1, Wm2, bm2, Wf1, bf1, Wf2, bf2,
    )
    nc = build_program(CH, nz_bm2, nz_bf2)
    res = run_bass_kernel_spmd(nc, in_maps, list(range(NCORES)))
    out = np.concatenate(
        [np.asarray(res.results[c]["out"]).reshape(RANGE, FD)
         for c in range(NCORES)], axis=0
    )[:N]
    return np.ascontiguousarray(out.astype(np.float32))
